# revision 36
# baseline (speedup 1.0000x reference)
"""Trainium2 Bass kernel for nn_AdaptivePoolingClassifier (8 NeuronCores).

Math: the reference MLP is linear up to its single ReLU, so W1..W3 fold
into one 128x128 matrix on the host:
    h   = relu(x @ Wc^T + bc)       Wc = W3 W2 W1 ; bc = W3(W2 b1+b2)+b3
    p   = h @ W4^T + b4
    out = sum_n p * softmax(alpha*p, axis=n)

Device computes (rows sharded 8 ways, all-bf16 inputs):
    pt  = h @ (diag(alpha) W4)^T        # = alpha*(p - b4), [rows, 5]
    den_partial = sum_rows exp(pt) ; num_partial = sum_rows pt*exp(pt)
Host finishes: out_o = num_o/(alpha_o*den_o) + b4_o (the softmax is
invariant to the per-column constant factor exp(-alpha_o*b4_o); a zero
bc is dropped entirely, a nonzero bc enters as a rank-1 matmul).

Layout: x is transposed on the host to [128(feat), rows] so features sit
on SBUF partitions for the folded matmul; layer-4 uses h chunks as the
matmul *stationary* operand so pt lands rows-on-partitions, making the
pooling full-width [128, 240] ops instead of lane-starved [5, n] ones.
ReLU is split between ACT and DVE by column range (chunk-aligned).
"""

import numpy as np
import ml_dtypes

from concourse import bacc, mybir, tile
from concourse.bass_utils import run_bass_kernel_spmd

N_CORES = 8
N_ROWS = 200000
F = 128
OUT = 5

ROWS_PAD = 200704            # 8 * 25088 ; 25088 = 512 + 3*8192
RPC = ROWS_PAD // N_CORES    # rows per core = 25088
T0 = 512                     # first tile (rides with the constants)
TILE = 1024                  # steady-state compute tile
GROUP = 2048                 # rows per steady-state DMA (512 KiB bf16)
N_GROUPS = (RPC - T0) // GROUP  # 12
CHUNK = 128                  # rows per layer-4 matmul (stationary M)
N_CHUNKS = RPC // CHUNK      # 196
SLOTS = 48                   # pt chunks per pooling batch (one PSUM bank)
N_BATCH = (N_CHUNKS + SLOTS - 1) // SLOTS  # 5
HALF = SLOTS // 2            # slots per PE-reduction half
HALF_COLS = HALF * OUT       # 120 output partitions per reduce matmul

F32 = mybir.dt.float32
BF16 = mybir.dt.bfloat16
AF = mybir.ActivationFunctionType
ALU = mybir.AluOpType


def build_bass(has_bias=False):
    nc = bacc.Bacc()

    # xt carries [WcT | W4aT | bc_row | x^T shard] per core (all bf16)
    CONST_COLS = (2 * F + OUT) if has_bias else (F + OUT)
    xt_ext = nc.declare_dram_parameter(
        "xt", [F, CONST_COLS + RPC], BF16, isOutput=False
    )
    out_ext = nc.declare_dram_parameter(
        "out", [F, 2, N_BATCH, OUT], F32, isOutput=True
    )

    with tile.TileContext(nc) as tc:
        with (
            tc.tile_pool(name="scratch", bufs=1) as scratch,
            tc.tile_pool(name="xfirst", bufs=1) as xfirst_pool,
            tc.tile_pool(name="accs", bufs=1) as accs,
            tc.tile_pool(name="xin", bufs=5) as xin,
            tc.tile_pool(name="hbufl", bufs=4) as hbufl,
            tc.tile_pool(name="hbufr", bufs=4) as hbufr,
            tc.tile_pool(name="ebuf", bufs=3) as ebuf,
            tc.tile_pool(name="ps_h", bufs=3, space="PSUM") as ps_h,
            tc.tile_pool(name="ps_p", bufs=2, space="PSUM") as ps_p,
        ):
            # per-batch partial sums, written slice-wise by DVE reduces
            parts = accs.tile([F, 2, N_BATCH, OUT], F32)

            # first DMA: all constants + tile 0 in one transfer
            xfirst = xfirst_pool.tile([F, CONST_COLS + T0], BF16)
            nc.sync.dma_start(out=xfirst[:], in_=xt_ext[:, : CONST_COLS + T0])
            wct = xfirst[:, :F]
            w4at = xfirst[:, F : F + OUT]
            nc.tensor.ldweights(wct)  # PE observes the const DMA early
            if has_bias:
                # bc enters h3 as a rank-1 accumulating matmul:
                # lhsT = bc row [1, 128], rhs = ones [1, TILE]
                bc_row = xfirst[0:1, F + OUT : F + OUT + F]
                ones = scratch.tile([1, TILE], BF16)
                nc.vector.memset(ones[:], 1.0)

            state = {"pp": None, "chunk": 0}

            def flush_batch(n_slots):
                bi = (state["chunk"] - 1) // SLOTS
                pp = state["pp"]
                sl = slice(0, n_slots)
                e_b = ebuf.tile([F, OUT, SLOTS], F32, tag="e_b")
                pe_b = ebuf.tile([F, OUT, SLOTS], F32, tag="pe_b")
                nc.scalar.activation(e_b[:, :, sl], pp[:, :, sl], AF.Exp)
                nc.vector.tensor_tensor(
                    pe_b[:, :, sl], pp[:, :, sl], e_b[:, :, sl], ALU.mult
                )
                nc.vector.tensor_reduce(
                    parts[:, 0, bi, :], e_b[:, :, sl],
                    mybir.AxisListType.X, ALU.add,
                )
                nc.vector.tensor_reduce(
                    parts[:, 1, bi, :], pe_b[:, :, sl],
                    mybir.AxisListType.X, ALU.add,
                )

            def do_tile(rhs, rows, no_act=False):
                n_ch = rows // CHUNK
                a_ch = 0 if no_act else max(1, n_ch // 2)
                d_ch = n_ch - a_ch           # leading chunks on DVE
                d_cols = d_ch * CHUNK
                h3p = ps_h.tile([F, TILE], F32, tag="h3p")
                # one matmul per 512-col PSUM bank (f32 free-dim limit)
                for c0 in range(0, rows, 512):
                    cw = min(512, rows - c0)
                    nc.tensor.matmul(
                        h3p[:, c0 : c0 + cw], wct, rhs[:, c0 : c0 + cw],
                        start=True, stop=not has_bias,
                    )
                    if has_bias:
                        nc.tensor.matmul(
                            h3p[:, c0 : c0 + cw], bc_row, ones[:, :cw],
                            start=False, stop=True,
                        )
                hl = hbufl.tile([F, 4 * CHUNK], BF16, tag="hl")
                hr = hbufr.tile([F, 8 * CHUNK], BF16, tag="hr")
                nc.vector.tensor_scalar_max(
                    hr[:, :d_cols], h3p[:, :d_cols], 0.0
                )
                if a_ch:
                    nc.scalar.activation(
                        hl[:, : rows - d_cols], h3p[:, d_cols:rows], AF.Relu
                    )
                for j in range(n_ch):
                    c = state["chunk"]
                    s = c % SLOTS
                    if s == 0:
                        state["pp"] = ps_p.tile([F, OUT, SLOTS], F32, tag="pp", name="pp")
                    if j < d_ch:
                        lhs = hr[:, j * CHUNK : (j + 1) * CHUNK]
                    else:
                        lhs = hl[:, (j - d_ch) * CHUNK : (j - d_ch + 1) * CHUNK]
                    nc.tensor.matmul(
                        state["pp"][:, :, s], lhs, w4at,
                        start=True, stop=True,
                    )
                    state["chunk"] = c + 1
                    if s == SLOTS - 1 or state["chunk"] == N_CHUNKS:
                        flush_batch(s + 1)

            tcount = 0
            do_tile(xfirst[:, CONST_COLS:], T0, no_act=True)
            tcount += 1
            for g in range(N_GROUPS):
                c0 = CONST_COLS + T0 + g * GROUP
                xt_t = xin.tile([F, GROUP], BF16)
                nc.sync.dma_start(out=xt_t[:], in_=xt_ext[:, c0 : c0 + GROUP])
                for s in range(GROUP // TILE):
                    do_tile(
                        xt_t[:, s * TILE : (s + 1) * TILE], TILE,
                        no_act=tcount < 2,
                    )
                    tcount += 1

            nc.sync.dma_start(out=out_ext[:], in_=parts[:])

    nc.finalize()
    return nc


_CACHED = {}
TRACE = False
LAST = {}


def kernel(x, W1, b1, W2, b2, W3, b3, W4, b4, alpha):
    f64 = np.float64
    x2 = np.asarray(x, np.float32).reshape(N_ROWS, F)
    W1, b1, W2, b2, W3, b3, W4, b4, alpha = [
        np.asarray(a, f64) for a in (W1, b1, W2, b2, W3, b3, W4, b4, alpha)
    ]

    # fold the linear layers (exact in f64)
    Wc = W3 @ W2 @ W1
    bc = W3 @ (W2 @ b1 + b2) + b3
    alpha_safe = np.where(np.abs(alpha) < 1e-12, 1e-12, alpha)
    W4a = alpha_safe[:, None] * W4

    # pad rows to 8*25088 with zeros; contribution removed on the host
    n_pad = ROWS_PAD - N_ROWS
    xp = np.concatenate([x2, np.zeros((n_pad, F), np.float32)], axis=0)
    xT = np.ascontiguousarray(xp.T).astype(ml_dtypes.bfloat16)  # [128, ROWS_PAD]

    has_bias = bool(np.any(bc != 0.0))
    key = ("nc", has_bias)
    if key not in _CACHED:
        _CACHED[key] = build_bass(has_bias)
    nc = _CACHED[key]

    wct_np = np.ascontiguousarray(Wc.T).astype(ml_dtypes.bfloat16)
    w4at_np = np.ascontiguousarray(W4a.T).astype(ml_dtypes.bfloat16)
    parts_list = [wct_np, w4at_np]
    if has_bias:
        bc_blk = np.zeros((F, F), np.float32)
        bc_blk[0, :] = bc.astype(np.float32)
        parts_list.append(bc_blk.astype(ml_dtypes.bfloat16))
    consts_np = np.concatenate(parts_list, axis=1)

    in_maps = []
    for c in range(N_CORES):
        shard = xT[:, c * RPC : (c + 1) * RPC]
        in_maps.append(
            {"xt": np.ascontiguousarray(np.concatenate([consts_np, shard], axis=1))}
        )

    res = run_bass_kernel_spmd(
        nc, in_maps, core_ids=list(range(N_CORES)), trace=TRACE
    )
    LAST["res"] = res
    outs = np.stack(
        [np.asarray(r["out"], f64) for r in res.results]
    )  # [8, F, 2, NB, OUT]
    den = outs[:, :, 0].sum(axis=(0, 1, 2))  # [5]
    num = outs[:, :, 1].sum(axis=(0, 1, 2))  # [5]

    # remove the zero-pad rows' contribution (each pad row: h0 = relu(bc))
    h0 = np.maximum(bc, 0.0)
    pt0 = W4a @ h0
    den -= n_pad * np.exp(pt0)
    num -= n_pad * pt0 * np.exp(pt0)

    out = num / (alpha_safe * den) + b4
    return out[None, :].astype(np.float32)


# revision 37
# speedup vs baseline: 1.0302x; 1.0302x over previous
"""Trainium2 Bass kernel for nn_AdaptivePoolingClassifier (8 NeuronCores).

Math: the reference MLP is linear up to its single ReLU, so W1..W3 fold
into one 128x128 matrix on the host:
    h   = relu(x @ Wc^T + bc)       Wc = W3 W2 W1 ; bc = W3(W2 b1+b2)+b3
    p   = h @ W4^T + b4
    out = sum_n p * softmax(alpha*p, axis=n)

Device computes (rows sharded 8 ways, all-bf16 inputs):
    pt  = h @ (diag(alpha) W4)^T        # = alpha*(p - b4), [rows, 5]
    den_partial = sum_rows exp(pt) ; num_partial = sum_rows pt*exp(pt)
Host finishes: out_o = num_o/(alpha_o*den_o) + b4_o (the softmax is
invariant to the per-column constant factor exp(-alpha_o*b4_o); a zero
bc is dropped entirely, a nonzero bc enters as a rank-1 matmul).

Layout: x is transposed on the host to [128(feat), rows] so features sit
on SBUF partitions for the folded matmul; layer-4 uses h chunks as the
matmul *stationary* operand so pt lands rows-on-partitions, making the
pooling full-width [128, 240] ops instead of lane-starved [5, n] ones.
ReLU is split between ACT and DVE by column range (chunk-aligned).
"""

import numpy as np
import ml_dtypes

from concourse import bacc, mybir, tile
from concourse.bass_utils import run_bass_kernel_spmd

N_CORES = 8
N_ROWS = 200000
F = 128
OUT = 5

ROWS_PAD = 200704            # 8 * 25088 ; 25088 = 512 + 3*8192
RPC = ROWS_PAD // N_CORES    # rows per core = 25088
T0 = 512                     # first tile (rides with the constants)
TILE = 1024                  # steady-state compute tile
GROUP = 2048                 # rows per steady-state DMA (512 KiB bf16)
N_GROUPS = (RPC - T0) // GROUP  # 12
CHUNK = 128                  # rows per layer-4 matmul (stationary M)
N_CHUNKS = RPC // CHUNK      # 196
SLOTS = 48                   # pt chunks per pooling batch (one PSUM bank)
N_BATCH = (N_CHUNKS + SLOTS - 1) // SLOTS  # 5
HALF = SLOTS // 2            # slots per PE-reduction half
HALF_COLS = HALF * OUT       # 120 output partitions per reduce matmul

F32 = mybir.dt.float32
BF16 = mybir.dt.bfloat16
AF = mybir.ActivationFunctionType
ALU = mybir.AluOpType


def build_bass(has_bias=False):
    nc = bacc.Bacc()

    # xt carries [WcT | W4aT | bc_row | x^T shard] per core (all bf16)
    CONST_COLS = (2 * F + OUT) if has_bias else (F + OUT)
    xt_ext = nc.declare_dram_parameter(
        "xt", [F, CONST_COLS + RPC], BF16, isOutput=False
    )
    out_ext = nc.declare_dram_parameter(
        "out", [F, 2, N_BATCH, OUT], F32, isOutput=True
    )

    with tile.TileContext(nc) as tc:
        with (
            tc.tile_pool(name="scratch", bufs=1) as scratch,
            tc.tile_pool(name="xfirst", bufs=1) as xfirst_pool,
            tc.tile_pool(name="accs", bufs=1) as accs,
            tc.tile_pool(name="xin", bufs=5) as xin,
            tc.tile_pool(name="hbufl", bufs=4) as hbufl,
            tc.tile_pool(name="hbufr", bufs=4) as hbufr,
            tc.tile_pool(name="ebuf", bufs=3) as ebuf,
            tc.tile_pool(name="ps_h", bufs=3, space="PSUM") as ps_h,
            tc.tile_pool(name="ps_p", bufs=2, space="PSUM") as ps_p,
        ):
            # per-batch partial sums, written slice-wise by DVE reduces
            parts = accs.tile([F, 2, N_BATCH, OUT], F32)

            # first DMA: all constants + tile 0 in one transfer
            xfirst = xfirst_pool.tile([F, CONST_COLS + T0], BF16)
            nc.sync.dma_start(out=xfirst[:], in_=xt_ext[:, : CONST_COLS + T0])
            wct = xfirst[:, :F]
            w4at = xfirst[:, F : F + OUT]
            nc.tensor.ldweights(wct)  # PE observes the const DMA early
            if has_bias:
                # bc enters h3 as a rank-1 accumulating matmul:
                # lhsT = bc row [1, 128], rhs = ones [1, TILE]
                bc_row = xfirst[0:1, F + OUT : F + OUT + F]
                ones = scratch.tile([1, TILE], BF16)
                nc.vector.memset(ones[:], 1.0)

            state = {"pp": None, "chunk": 0}

            def flush_batch(n_slots):
                bi = (state["chunk"] - 1) // SLOTS
                pp = state["pp"]
                sl = slice(0, n_slots)
                e_b = ebuf.tile([F, OUT, SLOTS], F32, tag="e_b")
                pe_b = ebuf.tile([F, OUT, SLOTS], F32, tag="pe_b")
                nc.scalar.activation(e_b[:, :, sl], pp[:, :, sl], AF.Exp)
                nc.vector.tensor_tensor(
                    pe_b[:, :, sl], pp[:, :, sl], e_b[:, :, sl], ALU.mult
                )
                nc.vector.tensor_reduce(
                    parts[:, 0, bi, :], e_b[:, :, sl],
                    mybir.AxisListType.X, ALU.add,
                )
                nc.vector.tensor_reduce(
                    parts[:, 1, bi, :], pe_b[:, :, sl],
                    mybir.AxisListType.X, ALU.add,
                )

            def do_tile(rhs, rows, no_act=False):
                n_ch = rows // CHUNK
                a_ch = 0 if no_act else max(1, n_ch // 2)
                a_cols = a_ch * CHUNK
                h3p = ps_h.tile([F, TILE], F32, tag="h3p")
                # one matmul per 512-col PSUM bank (f32 free-dim limit)
                for c0 in range(0, rows, 512):
                    cw = min(512, rows - c0)
                    nc.tensor.matmul(
                        h3p[:, c0 : c0 + cw], wct, rhs[:, c0 : c0 + cw],
                        start=True, stop=not has_bias,
                    )
                    if has_bias:
                        nc.tensor.matmul(
                            h3p[:, c0 : c0 + cw], bc_row, ones[:, :cw],
                            start=False, stop=True,
                        )
                hl = hbufl.tile([F, 4 * CHUNK], BF16, tag="hl")
                hr = hbufr.tile([F, 8 * CHUNK], BF16, tag="hr")
                if a_ch:
                    nc.scalar.activation(
                        hl[:, :a_cols], h3p[:, :a_cols], AF.Relu
                    )
                nc.vector.tensor_scalar_max(
                    hr[:, : rows - a_cols], h3p[:, a_cols:rows], 0.0
                )
                for j in range(n_ch):
                    c = state["chunk"]
                    s = c % SLOTS
                    if s == 0:
                        state["pp"] = ps_p.tile([F, OUT, SLOTS], F32, tag="pp", name="pp")
                    if j < a_ch:
                        lhs = hl[:, j * CHUNK : (j + 1) * CHUNK]
                    else:
                        lhs = hr[:, (j - a_ch) * CHUNK : (j - a_ch + 1) * CHUNK]
                    nc.tensor.matmul(
                        state["pp"][:, :, s], lhs, w4at,
                        start=True, stop=True,
                    )
                    state["chunk"] = c + 1
                    if s == SLOTS - 1 or state["chunk"] == N_CHUNKS:
                        flush_batch(s + 1)

            tcount = 0
            do_tile(xfirst[:, CONST_COLS:], T0, no_act=True)
            tcount += 1
            for g in range(N_GROUPS):
                c0 = CONST_COLS + T0 + g * GROUP
                xt_t = xin.tile([F, GROUP], BF16)
                nc.sync.dma_start(out=xt_t[:], in_=xt_ext[:, c0 : c0 + GROUP])
                for s in range(GROUP // TILE):
                    do_tile(
                        xt_t[:, s * TILE : (s + 1) * TILE], TILE,
                        no_act=tcount < 2,
                    )
                    tcount += 1

            nc.sync.dma_start(out=out_ext[:], in_=parts[:])

    nc.finalize()
    return nc


_CACHED = {}
TRACE = False
LAST = {}


def kernel(x, W1, b1, W2, b2, W3, b3, W4, b4, alpha):
    f64 = np.float64
    x2 = np.asarray(x, np.float32).reshape(N_ROWS, F)
    W1, b1, W2, b2, W3, b3, W4, b4, alpha = [
        np.asarray(a, f64) for a in (W1, b1, W2, b2, W3, b3, W4, b4, alpha)
    ]

    # fold the linear layers (exact in f64)
    Wc = W3 @ W2 @ W1
    bc = W3 @ (W2 @ b1 + b2) + b3
    alpha_safe = np.where(np.abs(alpha) < 1e-12, 1e-12, alpha)
    W4a = alpha_safe[:, None] * W4

    # pad rows to 8*25088 with zeros; contribution removed on the host
    n_pad = ROWS_PAD - N_ROWS
    xp = np.concatenate([x2, np.zeros((n_pad, F), np.float32)], axis=0)
    xT = np.ascontiguousarray(xp.T).astype(ml_dtypes.bfloat16)  # [128, ROWS_PAD]

    has_bias = bool(np.any(bc != 0.0))
    key = ("nc", has_bias)
    if key not in _CACHED:
        _CACHED[key] = build_bass(has_bias)
    nc = _CACHED[key]

    wct_np = np.ascontiguousarray(Wc.T).astype(ml_dtypes.bfloat16)
    w4at_np = np.ascontiguousarray(W4a.T).astype(ml_dtypes.bfloat16)
    parts_list = [wct_np, w4at_np]
    if has_bias:
        bc_blk = np.zeros((F, F), np.float32)
        bc_blk[0, :] = bc.astype(np.float32)
        parts_list.append(bc_blk.astype(ml_dtypes.bfloat16))
    consts_np = np.concatenate(parts_list, axis=1)

    in_maps = []
    for c in range(N_CORES):
        shard = xT[:, c * RPC : (c + 1) * RPC]
        in_maps.append(
            {"xt": np.ascontiguousarray(np.concatenate([consts_np, shard], axis=1))}
        )

    res = run_bass_kernel_spmd(
        nc, in_maps, core_ids=list(range(N_CORES)), trace=TRACE
    )
    LAST["res"] = res
    outs = np.stack(
        [np.asarray(r["out"], f64) for r in res.results]
    )  # [8, F, 2, NB, OUT]
    den = outs[:, :, 0].sum(axis=(0, 1, 2))  # [5]
    num = outs[:, :, 1].sum(axis=(0, 1, 2))  # [5]

    # remove the zero-pad rows' contribution (each pad row: h0 = relu(bc))
    h0 = np.maximum(bc, 0.0)
    pt0 = W4a @ h0
    den -= n_pad * np.exp(pt0)
    num -= n_pad * pt0 * np.exp(pt0)

    out = num / (alpha_safe * den) + b4
    return out[None, :].astype(np.float32)
